# revision 4
# baseline (speedup 1.0000x reference)
"""Group-equivariant conv (folds to dense 128->128ch 3x3 conv, pad=1) on 8 trn2 cores.

Strategy: data-parallel over batch (2 images/core). The G^2-shifted group conv
is expanded on host (pure index shuffle, no FLOPs) into a dense [128,128,3,3]
weight. On device each image is laid out channel-on-partition as a zero-padded
flattened plane [128, 130*130]; the 3x3 conv is 9 PSUM-accumulated matmuls per
output chunk, where tap (dy,dx)'s rhs is just a constant-offset slice of the
flattened plane. fp32r matmul dtype = full PE rate at fp32 storage.
"""

import sys

for _p in ("/opt/trn_rl_repo",):
    if _p not in sys.path:
        sys.path.insert(0, _p)

from contextlib import ExitStack

import numpy as np

import concourse.bacc as bacc
import concourse.mybir as mybir
import concourse.tile as tile
from concourse.bass_utils import run_bass_kernel_spmd

NCORES = 8
B, C, H, W = 16, 128, 128, 128
BPC = B // NCORES           # images per core
S = W + 2                   # padded row stride
XCOLS = (H + 2) * S + 4     # padded plane + tail guard for last tap reads
CH = 3                      # output rows per PSUM chunk (N = 3*130 = 390 <= 512)
NBLK = 8                    # input row-blocks per image (DMA pipelining)
GRP = 8                     # chunks per PSUM group (8 banks)

F32 = mybir.dt.float32
F32R = mybir.dt.float32r


def _expand_weight(weight: np.ndarray) -> np.ndarray:
    """[32,32,4,3,3] -> lhsT layout [ci=128, tap=9, co=128] flattened [128, 1152]."""
    o, i, g, kh, kw = weight.shape
    gi = np.arange(g)
    shift = (gi[:, None] - gi[None, :]) % g            # [g, h]
    wb = weight[:, :, shift]                           # [o, i, g, h, kh, kw]
    wb = np.transpose(wb, (2, 0, 1, 3, 4, 5))          # [g, o, i, h, kh, kw]
    wb = wb.reshape(g * o, i * g, kh, kw)              # [co=128, ci=128, 3, 3]
    wt = np.transpose(wb, (1, 2, 3, 0))                # [ci, kh, kw, co]
    return np.ascontiguousarray(wt.reshape(C, 9 * C)).astype(np.float32)


def _chunks():
    out = []
    y = 0
    while y < H:
        rows = min(CH, H - y)
        out.append((y, rows))
        y += rows
    return out


def _build_body(ctx: ExitStack, tc: tile.TileContext, x_ap, wt_ap, out_ap):
    nc = tc.nc
    xpool = ctx.enter_context(tc.tile_pool(name="xp", bufs=1))
    wpool = ctx.enter_context(tc.tile_pool(name="wp", bufs=1))
    opool = ctx.enter_context(tc.tile_pool(name="op", bufs=2))
    ppool = ctx.enter_context(tc.tile_pool(name="pp", bufs=8, space="PSUM"))

    wt = wpool.tile([C, 9 * C], F32R, name="wt_sb")
    nc.sync.dma_start(out=wt[:], in_=wt_ap[:])

    xbufs = []
    for i in range(BPC):
        xb = xpool.tile([C, XCOLS], F32R, name=f"xb{i}", tag=f"xb{i}")
        xbufs.append(xb)
        # Zero only the pad cells once; interior DMAs never touch them.
        # (memset can't encode float32r — bitcast the APs to plain f32.)
        nc.vector.memset(xb[:, 0:S].bitcast(F32), 0.0)            # top pad row
        nc.vector.memset(xb[:, (H + 1) * S:XCOLS].bitcast(F32), 0.0)  # bottom row + guard
        pairs = xb[:, S - 1:S - 1 + (H + 1) * S].rearrange(
            "p (r s) -> p r s", s=S)[:, :, 0:2]                   # col pads (row ends)
        nc.vector.memset(pairs.bitcast(F32), 0.0)

    chunks = _chunks()
    groups = [chunks[i:i + GRP] for i in range(0, len(chunks), GRP)]
    RB = H // NBLK

    for img in range(BPC):
        xb = xbufs[img]
        xview = xb[:, 0:(H + 2) * S].rearrange("p (r s) -> p r s", s=S)
        for blk in range(NBLK):
            r0 = blk * RB
            nc.sync.dma_start(
                out=xview[:, 1 + r0:1 + r0 + RB, 1:1 + W],
                in_=x_ap[img, :, r0:r0 + RB, :],
            )

        for grp in groups:
            g_y0 = grp[0][0]
            g_rows = sum(r for _, r in grp)
            psums = [ppool.tile([C, 512], F32, name="ps", tag="ps") for _ in grp]
            for t in range(9):
                dy, dx = divmod(t, 3)
                wslice = wt[:, t * C:(t + 1) * C]
                for pt, (y, rows) in zip(psums, grp):
                    n = rows * S
                    off = (y + dy) * S + dx
                    nc.tensor.matmul(
                        pt[:, 0:n], wslice, xb[:, off:off + n],
                        start=(t == 0), stop=(t == 8),
                    )
            stage = opool.tile([C, g_rows * W], F32, name="stage", tag="stage")
            col = 0
            for pt, (y, rows) in zip(psums, grp):
                src = pt[:, 0:rows * S].rearrange("p (r s) -> p r s", s=S)[:, :, 0:W]
                dst = stage[:, col:col + rows * W].rearrange("p (r s) -> p r s", s=W)
                nc.vector.tensor_copy(dst, src)
                col += rows * W
            nc.sync.dma_start(
                out=out_ap[img, :, g_y0:g_y0 + g_rows, :],
                in_=stage[:, 0:g_rows * W],
            )


_NC_CACHE = None


def _get_nc():
    global _NC_CACHE
    if _NC_CACHE is None:
        nc = bacc.Bacc("TRN2", target_bir_lowering=False, debug=False)
        x_ap = nc.dram_tensor("x", [BPC, C, H, W], F32R, kind="ExternalInput").ap()
        wt_ap = nc.dram_tensor("wt", [C, 9 * C], F32R, kind="ExternalInput").ap()
        out_ap = nc.dram_tensor("out", [BPC, C, H, W], F32, kind="ExternalOutput").ap()
        with tile.TileContext(nc) as tc:
            with ExitStack() as ctx:
                _build_body(ctx, tc, x_ap, wt_ap, out_ap)
        nc.compile()
        _NC_CACHE = nc
    return _NC_CACHE


def _run(x: np.ndarray, weight: np.ndarray, trace: bool = False, **kw):
    x = np.ascontiguousarray(np.asarray(x, dtype=np.float32))
    wt = _expand_weight(np.asarray(weight, dtype=np.float32))
    nc = _get_nc()
    in_maps = [
        {"x": x[c * BPC:(c + 1) * BPC], "wt": wt} for c in range(NCORES)
    ]
    res = run_bass_kernel_spmd(nc, in_maps, list(range(NCORES)), trace=trace, **kw)
    out = np.concatenate([res.results[c]["out"] for c in range(NCORES)], axis=0)
    return out, res


def kernel(x: np.ndarray, weight: np.ndarray) -> np.ndarray:
    out, _ = _run(x, weight)
    return out
